# revision 1
# baseline (speedup 1.0000x reference)
"""Trainium2 Bass kernel for nn_Discriminator (histogram_binning / ridge).

Math (reference):
  For each batch n (N=32): interpolate P=128 points into M=(P-1)*181=22987
  line points (x,y,w); splat Gaussians g_x[m,s]=exp(-(x_m-s)^2/(2 w_m)),
  g_y[m,t]; canvas = g_x^T @ g_y  [128,128]; line = tanh(canvas);
  loss = sum(BCE(line, img))/N + sum(poly_sqrt(seg_len^2))/N.

Device strategy (data-parallel over N, 4 batches per core, 8 cores):
  The Gaussian exponent arg[m,s] = c2[m]*s'^2 + c1[m]*s' + c0[m] (s'=s-64)
  is computed on the TensorEngine as a K=24 bf16 matmul: the basis rows
  (s'^2 split into two exactly-representable bf16 rows, s', 1) are exact,
  and each coefficient is split into 3 bf16 levels (~25-bit mantissa).
  A block-diagonal basis computes the x-arg and y-arg in one matmul
  ([24,128] lhsT x [24,256] rhs -> [128m, 256]). ScalarE applies one Exp
  per element (PSUM->SBUF, bf16 out), and the canvas accumulates 180
  chunk matmuls (K=128, bf16) in PSUM. tanh/log/BCE epilogue per batch,
  free-dim reduced on DVE; final partition sums on host.
"""
import sys
import types
import numpy as np
import ml_dtypes

# ---------------------------------------------------------------- constants
IMG = 128          # image size S
P = 128            # points per batch
N = 32             # batch
CMP = int(IMG * np.sqrt(2))            # 181
M = (P - 1) * CMP                      # 22987 line points per batch
NCHUNK = (M + 127) // 128              # 180
MPAD = NCHUNK * 128                    # 23040
NCORES = 8
NB = N // NCORES                       # 4 batches per core
GRP = 6                                # arg chunks per Exp instruction
NGRP = NCHUNK // GRP                   # 45
CENTER = 64.0

_d = np.arange(-IMG + 1, IMG)
X0 = float((_d ** 2 + (_d ** 2).T).mean().astype(np.float32))
C0 = float(X0 ** 0.5)
C1 = float(X0 ** (-0.5) / 2.0)
C2 = float(-(X0 ** (-1.5) / 8.0))
C3 = float(X0 ** (-2.5) / 16.0)

_BF = ml_dtypes.bfloat16

# XLA:CPU f32 tanh returns exactly 1.0 for x >= this (empirical, bit-exact);
# the reference's clip(log(1-line), -100) then yields -100 on those pixels.
TANH_SAT = float(np.uint32(1090516548).view(np.float32))  # 7.9988117
ULP_BELOW_1 = 5.960464477539063e-08  # 1 - nextafter(1, 0) in f32


def _install_ntff_hook():
    """bass_utils wants antenv.axon_hooks for trace=True under axon; the image
    lacks it. Provide it, backed by the ctypes shim in trn_agent_boot."""
    if 'antenv.axon_hooks' in sys.modules:
        return
    mod = types.ModuleType('antenv.axon_hooks')
    _h = [None]
    mod.set_axon_ntff_profile_hook = lambda h: _h.__setitem__(0, h)
    mod.get_axon_ntff_profile_hook = lambda: _h[0]
    sys.modules['antenv.axon_hooks'] = mod
    try:
        from trn_agent_boot.trn_boot import _ntff_profile_via_ctypes
        mod.set_axon_ntff_profile_hook(
            _ntff_profile_via_ctypes('/opt/axon/libaxon_pjrt.so'))
    except Exception:
        pass


_install_ntff_hook()

import concourse.bass as bass          # noqa: E402
import concourse.tile as tile          # noqa: E402
from concourse import bacc, mybir      # noqa: E402
from concourse.bass_utils import run_bass_kernel_spmd  # noqa: E402

dt = mybir.dt
AF = mybir.ActivationFunctionType
ALU = mybir.AluOpType


# ---------------------------------------------------------------- host prep
def _bf16_split3(x):
    h = x.astype(_BF).astype(np.float64)
    m = (x - h).astype(_BF).astype(np.float64)
    l = (x - h - m).astype(_BF).astype(np.float64)
    return h, m, l


def _build_q24():
    """Block-diagonal exact bf16 basis, zero-padded to K=128 rows (the PE's
    HAM clock-gate only counts full-K matmuls as activity)."""
    sprime = np.arange(IMG, dtype=np.float64) - CENTER
    s2 = sprime ** 2
    s2h = s2.astype(_BF).astype(np.float64)
    s2l = s2 - s2h
    qrows = [s2h, s2l, sprime, np.ones(IMG)]
    q = np.zeros((128, 2 * IMG))
    for base, off in ((0, 0), (12, IMG)):
        for lvl in range(3):
            for j in range(4):
                q[base + lvl * 4 + j, off:off + IMG] = qrows[j]
    return q.astype(_BF)


def _build_f24(points):
    """points [N, P, 3] float -> F [N, 24, MPAD] bf16 coefficient rows."""
    pts = np.asarray(points, np.float64)
    t = (np.arange(CMP, dtype=np.float64) / CMP)[None, None, :, None]  # [1,1,CMP,1]
    a = pts[:, :-1, None, :]                                           # [N,P-1,1,3]
    b = pts[:, 1:, None, :]
    lp = (1.0 - t) * a + t * b                                         # [N,P-1,CMP,3]
    lp = lp.reshape(N, M, 3)
    x = lp[..., 0] - CENTER
    y = lp[..., 1] - CENTER
    invw = 1.0 / lp[..., 2]
    c2 = -0.5 * invw
    c1x = x * invw
    c0x = -0.5 * x * x * invw
    c1y = y * invw
    c0y = -0.5 * y * y * invw

    F = np.zeros((N, 128, MPAD))
    for base, c1_, c0_ in ((0, c1x, c0x), (12, c1y, c0y)):
        splits = [_bf16_split3(c2), _bf16_split3(c2),
                  _bf16_split3(c1_), _bf16_split3(c0_)]
        for lvl in range(3):
            for j in range(4):
                F[:, base + lvl * 4 + j, :M] = splits[j][lvl]
    # padding m in [M, MPAD): force arg_x = arg_y = -50 -> g ~ 0
    F[:, 3, M:] = -50.0
    F[:, 15, M:] = -50.0
    return F.astype(_BF)


# ---------------------------------------------------------------- device
def _build_nc():
    nc = bacc.Bacc("TRN2", target_bir_lowering=False, debug=False,
                   enable_asserts=False, num_devices=NCORES)
    f_in = nc.dram_tensor("f24", [NB, 128, MPAD], dt.bfloat16,
                          kind="ExternalInput").ap()
    q_in = nc.dram_tensor("q24", [128, 2 * IMG], dt.bfloat16,
                          kind="ExternalInput").ap()
    img_in = nc.dram_tensor("img", [NB, IMG, IMG], dt.float32,
                            kind="ExternalInput").ap()
    ptsa_in = nc.dram_tensor("ptsa", [NB, P - 1, 2], dt.float32,
                             kind="ExternalInput").ap()
    ptsb_in = nc.dram_tensor("ptsb", [NB, P - 1, 2], dt.float32,
                             kind="ExternalInput").ap()
    out = nc.dram_tensor("out", [128, 2 * NB], dt.float32,
                         kind="ExternalOutput").ap()

    with tile.TileContext(nc) as tc:
        with tc.tile_pool(name="const", bufs=1) as const_pool, \
             tc.tile_pool(name="fpool", bufs=2) as fpool, \
             tc.tile_pool(name="gpool", bufs=3) as gpool, \
             tc.tile_pool(name="small", bufs=2) as small, \
             tc.tile_pool(name="canv", bufs=2) as canv_pool, \
             tc.tile_pool(name="epi", bufs=2) as epi, \
             tc.tile_pool(name="argps", bufs=2, space="PSUM") as argps, \
             tc.tile_pool(name="canps", bufs=2, space="PSUM") as canps:

            qt = const_pool.tile([128, 2 * IMG], dt.bfloat16)
            nc.sync.dma_start(qt[:], q_in[:])
            outsb = const_pool.tile([128, 2 * NB], dt.float32)
            nc.vector.memset(outsb[:], 0.0)
            m100 = const_pool.tile([128, IMG], dt.float32)
            nc.vector.memset(m100[:], -100.0)
            mant_mask = const_pool.tile([128, 1], dt.int32)
            nc.vector.memset(mant_mask[:], 0x007FFFFF)
            one_bits = const_pool.tile([128, 1], dt.int32)
            nc.vector.memset(one_bits[:], 0x3F800000)

            canvases = []
            for n in range(NB):
                ft = fpool.tile([128, MPAD], dt.bfloat16, name="ft")
                for sl in range(4):
                    w = MPAD // 4
                    nc.sync.dma_start(ft[:, sl * w:(sl + 1) * w],
                                      f_in[n][:, sl * w:(sl + 1) * w])

                canvas_ps = canps.tile([128, IMG], dt.float32, name="canvas_ps")
                for g in range(NGRP):
                    arg_ps = argps.tile([128, GRP * 2 * IMG], dt.float32,
                                        name="arg_ps")
                    for i in range(GRP):
                        ch = g * GRP + i
                        nc.tensor.matmul(
                            arg_ps[:, i * 2 * IMG:(i + 1) * 2 * IMG],
                            ft[:, ch * 128:(ch + 1) * 128], qt[:],
                            start=True, stop=True)
                    gxy = gpool.tile([128, GRP * 2 * IMG], dt.bfloat16,
                                     name="gxy")
                    nc.scalar.activation(gxy[:], arg_ps[:], AF.Exp)
                    for i in range(GRP):
                        ch = g * GRP + i
                        o = i * 2 * IMG
                        nc.tensor.matmul(
                            canvas_ps[:],
                            gxy[:, o:o + IMG], gxy[:, o + IMG:o + 2 * IMG],
                            start=(ch == 0), stop=(ch == NCHUNK - 1))

                canvas_sb = canv_pool.tile([128, IMG], dt.float32,
                                           name="canvas_sb", bufs=NB)
                nc.vector.tensor_copy(canvas_sb[:], canvas_ps[:])
                canvases.append(canvas_sb)

            # ---- epilogue: BCE terms (deferred; tanh phase then ln phase
            # so ACT table sets load at most once each)
            lines = []
            for n in range(NB):
                line = epi.tile([128, IMG], dt.float32, name="line", bufs=NB)
                nc.scalar.activation(line[:], canvases[n][:], AF.Tanh)
                lines.append(line)
            for n in range(NB):
                canvas_sb = canvases[n]
                line = lines[n]
                imgt = small.tile([128, IMG], dt.float32, name="imgt")
                nc.sync.dma_start(imgt[:], img_in[n])

                # The Ln LUT is inaccurate below ~1e-7, but line spans down to
                # 1e-38. Exact range reduction instead:
                #   ln(x) = ln(mant in [1,2)) + (bits - mant_bits) * ln2/2^23
                # (the int subtract isolates the exponent field exactly).
                xb = line[:].bitcast(dt.int32)
                mb = epi.tile([128, IMG], dt.int32, name="mb")
                nc.vector.tensor_scalar(mb[:], xb, mant_mask[:, 0:1],
                                        one_bits[:, 0:1],
                                        ALU.bitwise_and, ALU.bitwise_or)
                db = epi.tile([128, IMG], dt.int32, name="db")
                nc.vector.tensor_tensor(db[:], xb, mb[:], ALU.subtract)
                ef = epi.tile([128, IMG], dt.float32, name="ef")
                nc.vector.tensor_copy(ef[:], db[:])
                nc.vector.tensor_scalar(ef[:], ef[:],
                                        0.6931471805599453 / (1 << 23),
                                        None, ALU.mult)
                logp = epi.tile([128, IMG], dt.float32, name="logp")
                nc.scalar.activation(logp[:], mb[:].bitcast(dt.float32), AF.Ln)
                nc.vector.tensor_tensor(logp[:], logp[:], ef[:], ALU.add)
                maskt = epi.tile([128, IMG], dt.uint8, name="maskt")
                nc.vector.tensor_scalar(maskt[:], line[:], 1e-38, None,
                                        ALU.is_lt)
                nc.vector.copy_predicated(logp[:], maskt[:], m100[:])
                u = epi.tile([128, IMG], dt.float32, name="u")
                nc.vector.tensor_scalar(u[:], line[:], -1.0, 1.0,
                                        ALU.mult, ALU.add)
                nc.vector.tensor_scalar(u[:], u[:], ULP_BELOW_1, None,
                                        ALU.max)
                log1mp = epi.tile([128, IMG], dt.float32, name="log1mp")
                nc.scalar.activation(log1mp[:], u[:], AF.Ln)
                nc.vector.tensor_scalar(maskt[:], canvas_sb[:], TANH_SAT,
                                        None, ALU.is_ge)
                nc.vector.copy_predicated(log1mp[:], maskt[:], m100[:])
                diff = epi.tile([128, IMG], dt.float32, name="diff")
                nc.vector.tensor_tensor(diff[:], logp[:], log1mp[:],
                                        ALU.subtract)
                prod = epi.tile([128, IMG], dt.float32, name="prod")
                nc.vector.tensor_tensor(prod[:], imgt[:], diff[:], ALU.mult)
                tot = epi.tile([128, IMG], dt.float32, name="tot")
                nc.vector.tensor_tensor(tot[:], prod[:], log1mp[:], ALU.add)
                nc.vector.tensor_reduce(outsb[:, n:n + 1], tot[:],
                                        mybir.AxisListType.X, ALU.add)

                # ---- distance term
                ta = small.tile([P - 1, 2], dt.float32, name="ta")
                tb = small.tile([P - 1, 2], dt.float32, name="tb")
                nc.sync.dma_start(ta[:], ptsa_in[n])
                nc.sync.dma_start(tb[:], ptsb_in[n])
                dxy = epi.tile([P - 1, 2], dt.float32, name="dxy")
                nc.vector.tensor_tensor(dxy[:], tb[:], ta[:], ALU.subtract)
                nc.vector.tensor_tensor(dxy[:], dxy[:], dxy[:], ALU.mult)
                segsq = epi.tile([P - 1, 1], dt.float32, name="segsq")
                nc.vector.tensor_reduce(segsq[:], dxy[:],
                                        mybir.AxisListType.X, ALU.add)
                dx = epi.tile([P - 1, 1], dt.float32, name="dx")
                nc.vector.tensor_scalar(dx[:], segsq[:], -X0, None, ALU.add)
                poly = epi.tile([P - 1, 1], dt.float32, name="poly")
                nc.vector.tensor_scalar(poly[:], dx[:], C3, C2,
                                        ALU.mult, ALU.add)
                nc.vector.tensor_tensor(poly[:], poly[:], dx[:], ALU.mult)
                nc.vector.tensor_scalar(poly[:], poly[:], C1, None, ALU.add)
                nc.vector.tensor_tensor(poly[:], poly[:], dx[:], ALU.mult)
                nc.vector.tensor_scalar(outsb[:P - 1, NB + n:NB + n + 1],
                                        poly[:], C0, None, ALU.add)

            nc.sync.dma_start(out[:], outsb[:])
    nc.compile()
    return nc


_NC_CACHE = None


def _get_nc():
    global _NC_CACHE
    if _NC_CACHE is None:
        _NC_CACHE = _build_nc()
    return _NC_CACHE


def make_in_maps(points, img):
    points = np.asarray(points, np.float32)
    img = np.asarray(img, np.float32)
    f24 = _build_f24(points)                   # [N, 24, MPAD] bf16
    q24 = _build_q24()                         # [24, 256] bf16
    in_maps = []
    for c in range(NCORES):
        sl = slice(c * NB, (c + 1) * NB)
        pts = points[sl]
        in_maps.append({
            "f24": np.ascontiguousarray(f24[sl]),
            "q24": q24,
            "img": np.ascontiguousarray(img[sl]),
            "ptsa": np.ascontiguousarray(pts[:, :P - 1, 0:2]),
            "ptsb": np.ascontiguousarray(pts[:, 1:, 0:2]),
        })
    return in_maps


def combine_outputs(results):
    bce_tot = 0.0
    dist_tot = 0.0
    for r in results:
        o = np.asarray(r["out"], np.float64)
        bce_tot += o[:, :NB].sum()
        dist_tot += o[:P - 1, NB:].sum()
    return np.float32((dist_tot - bce_tot) / N)


def kernel(points, img, _trace=False, _trace_kwargs=None):
    nc = _get_nc()
    in_maps = make_in_maps(points, img)
    kw = {}
    if _trace:
        kw.update(trace=True, trace_cores=[0])
        if _trace_kwargs:
            kw.update(_trace_kwargs)
    res = run_bass_kernel_spmd(nc, in_maps, core_ids=list(range(NCORES)), **kw)
    out = combine_outputs(res.results)
    if _trace:
        return out, res
    return out



# revision 5
# speedup vs baseline: 2.4930x; 2.4930x over previous
"""Trainium2 Bass kernel for nn_Discriminator (histogram_binning / ridge).

Math (reference):
  For each batch n (N=32): interpolate P=128 points into M=(P-1)*181=22987
  line points (x,y,w); splat Gaussians g_x[m,s]=exp(-(x_m-s)^2/(2 w_m)),
  g_y[m,t]; canvas = g_x^T @ g_y  [128,128]; line = tanh(canvas);
  loss = sum(BCE(line, img))/N + sum(poly_sqrt(seg_len^2))/N.

Key optimization (adaptive segment resampling):
  The reference samples every segment at 181 points (one per ~0.37 px),
  but the splatted Gaussian has sigma = sqrt(w) >= 0.71 px; a trapezoid
  rule at spacing h needs only h <= ALPHA*sqrt(w) for aliasing error
  2*exp(-2*pi^2*w/h^2) (Poisson summation).  Per segment we place
  nseg+1 = ceil(len/(ALPHA*sqrt(w_min)))+1 trapezoid nodes spanning
  t in [0, 180/181]; interior amplitude r = 180/nseg, endpoint
  amplitude (r+1)/2 reproduces the reference's 181-term sum up to
  ~1e-3 relative.  This cuts M ~6.7x (23040 -> ~3400 per batch).
  Amplitudes fold into the Gaussian offset: c0x += ln(amp).

Device strategy (data-parallel over N, 4 batches per core, 8 cores;
batches assigned to the 4 slots by descending chunk count so every
slot's chunk count T_n is tight across cores):
  The Gaussian exponent arg[m,s] = c2[m]*s'^2 + c1[m]*s' + c0[m] (s'=s-64)
  is computed on the TensorEngine as a K=24 bf16 matmul (zero-padded to
  K=128): basis rows (s'^2 split into two exact bf16 rows, s', 1), each
  coefficient split into 3 bf16 levels (~25-bit mantissa).  A block-
  diagonal basis computes x-arg and y-arg in one matmul
  ([128,128m] lhsT x [128,256] rhs).  ScalarE applies Exp (PSUM->SBUF,
  bf16 out), and the canvas accumulates T_n chunk matmuls (K=128, bf16)
  in PSUM.  tanh/log/BCE epilogue per batch; final sums on host.
  Dead padding points use c0 = -200 so exp underflows to exactly 0.
"""
import sys
import types
import numpy as np
import ml_dtypes

# ---------------------------------------------------------------- constants
IMG = 128          # image size S
P = 128            # points per batch
N = 32             # batch
CMP = int(IMG * np.sqrt(2))            # 181
NCORES = 8
NB = N // NCORES                       # 4 batches per core
GRP = 6                                # arg chunks per Exp instruction
CENTER = 64.0
ALPHA = 3.0                            # resampling spacing, in sigmas

_d = np.arange(-IMG + 1, IMG)
X0 = float((_d ** 2 + (_d ** 2).T).mean().astype(np.float32))
C0 = float(X0 ** 0.5)
C1 = float(X0 ** (-0.5) / 2.0)
C2 = float(-(X0 ** (-1.5) / 8.0))
C3 = float(X0 ** (-2.5) / 16.0)

_BF = ml_dtypes.bfloat16

# XLA:CPU f32 tanh returns exactly 1.0 for x >= this (empirical, bit-exact);
# the reference's clip(log(1-line), -100) then yields -100 on those pixels.
TANH_SAT = float(np.uint32(1090516548).view(np.float32))  # 7.9988117
ULP_BELOW_1 = 5.960464477539063e-08  # 1 - nextafter(1, 0) in f32


def _install_ntff_hook():
    """bass_utils wants antenv.axon_hooks for trace=True under axon; the image
    lacks it. Provide it, backed by the ctypes shim in trn_agent_boot."""
    if 'antenv.axon_hooks' in sys.modules:
        return
    mod = types.ModuleType('antenv.axon_hooks')
    _h = [None]
    mod.set_axon_ntff_profile_hook = lambda h: _h.__setitem__(0, h)
    mod.get_axon_ntff_profile_hook = lambda: _h[0]
    sys.modules['antenv.axon_hooks'] = mod
    try:
        from trn_agent_boot.trn_boot import _ntff_profile_via_ctypes
        mod.set_axon_ntff_profile_hook(
            _ntff_profile_via_ctypes('/opt/axon/libaxon_pjrt.so'))
    except Exception:
        pass


_install_ntff_hook()

import concourse.bass as bass          # noqa: E402
import concourse.tile as tile          # noqa: E402
from concourse import bacc, mybir      # noqa: E402
from concourse.bass_utils import run_bass_kernel_spmd  # noqa: E402

dt = mybir.dt
AF = mybir.ActivationFunctionType
ALU = mybir.AluOpType


# ---------------------------------------------------------------- host prep
def _bf16_split3(x):
    h = x.astype(_BF).astype(np.float64)
    m = (x - h).astype(_BF).astype(np.float64)
    l = (x - h - m).astype(_BF).astype(np.float64)
    return h, m, l


def _build_q24():
    """Block-diagonal exact bf16 basis, zero-padded to K=128 rows (the PE's
    HAM clock-gate only counts full-K matmuls as activity)."""
    sprime = np.arange(IMG, dtype=np.float64) - CENTER
    s2 = sprime ** 2
    s2h = s2.astype(_BF).astype(np.float64)
    s2l = s2 - s2h
    qrows = [s2h, s2l, sprime, np.ones(IMG)]
    q = np.zeros((128, 2 * IMG))
    for base, off in ((0, 0), (12, IMG)):
        for lvl in range(3):
            for j in range(4):
                q[base + lvl * 4 + j, off:off + IMG] = qrows[j]
    return q.astype(_BF)


def _resample_batch(pts):
    """pts [P,3] f64 -> (xs, ys, ws, amps) trapezoid-node resampling of the
    reference's per-segment 181-point splat."""
    a = pts[:-1]                                   # [P-1, 3]
    b = pts[1:]
    tJ = (CMP - 1) / CMP
    e = a + (b - a) * tJ                           # last fine sample per seg
    seglen = np.sqrt(((b[:, :2] - a[:, :2]) ** 2).sum(-1)) * tJ
    wmin = np.minimum(a[:, 2], e[:, 2])
    nseg = np.clip(np.ceil(seglen / (ALPHA * np.sqrt(wmin))), 1,
                   CMP - 1).astype(int)
    xs_l, ys_l, ws_l, am_l = [], [], [], []
    for s in range(P - 1):
        ns = nseg[s]
        ti = np.arange(ns + 1) * (tJ / ns)
        r = (CMP - 1) / ns
        amp = np.full(ns + 1, r)
        amp[0] = amp[-1] = (r + 1) / 2
        xs_l.append(a[s, 0] + (b[s, 0] - a[s, 0]) * ti)
        ys_l.append(a[s, 1] + (b[s, 1] - a[s, 1]) * ti)
        ws_l.append(a[s, 2] + (b[s, 2] - a[s, 2]) * ti)
        am_l.append(amp)
    return (np.concatenate(xs_l), np.concatenate(ys_l),
            np.concatenate(ws_l), np.concatenate(am_l))


def _build_f24(xs, ys, ws, amps, width):
    """samples -> F [32, width] bf16 coefficient rows (padded with dead
    points whose exp underflows to exactly 0)."""
    m = len(xs)
    x = xs - CENTER
    y = ys - CENTER
    invw = 1.0 / ws
    c2 = -0.5 * invw
    c1x = x * invw
    c0x = -0.5 * x * x * invw + np.log(amps)
    c1y = y * invw
    c0y = -0.5 * y * y * invw

    F = np.zeros((32, width))
    for base, c1_, c0_ in ((0, c1x, c0x), (12, c1y, c0y)):
        splits = [_bf16_split3(c2), _bf16_split3(c2),
                  _bf16_split3(c1_), _bf16_split3(c0_)]
        for lvl in range(3):
            for j in range(4):
                F[base + lvl * 4 + j, :m] = splits[j][lvl]
    # dead padding: arg_x = arg_y = -200 -> exp == 0 exactly (f32 underflow)
    F[3, m:] = -200.0
    F[15, m:] = -200.0
    return F.astype(_BF)


def _schedule(points):
    """Resample all batches, assign them to (core, slot) so each slot's
    chunk count is tight. Returns per-slot widths Ts and per-core data."""
    points = np.asarray(points, np.float64)
    samples = [_resample_batch(points[b]) for b in range(N)]
    chunks = np.array([(len(s[0]) + 127) // 128 for s in samples])
    order = np.argsort(-chunks, kind='stable')
    Ts = []
    core_batches = [[] for _ in range(NCORES)]
    for n in range(NB):
        grp = order[n * NCORES:(n + 1) * NCORES]
        Ts.append(int(chunks[grp].max()))
        for c in range(NCORES):
            core_batches[c].append(int(grp[c]))
    return tuple(Ts), core_batches, samples


# ---------------------------------------------------------------- device
def _build_nc(Ts):
    nc = bacc.Bacc("TRN2", target_bir_lowering=False, debug=False,
                   enable_asserts=False, num_devices=NCORES)
    f_ins = [nc.dram_tensor(f"fs{n}", [32, Ts[n] * 128], dt.bfloat16,
                            kind="ExternalInput").ap()
             for n in range(NB)]
    q_in = nc.dram_tensor("q24", [128, 2 * IMG], dt.bfloat16,
                          kind="ExternalInput").ap()
    img_in = nc.dram_tensor("img", [NB, IMG, IMG], dt.float32,
                            kind="ExternalInput").ap()
    ptsa_in = nc.dram_tensor("ptsa", [NB, P - 1, 2], dt.float32,
                             kind="ExternalInput").ap()
    ptsb_in = nc.dram_tensor("ptsb", [NB, P - 1, 2], dt.float32,
                             kind="ExternalInput").ap()
    out = nc.dram_tensor("out", [128, 2 * NB], dt.float32,
                         kind="ExternalOutput").ap()

    with tile.TileContext(nc) as tc:
        with tc.tile_pool(name="const", bufs=1) as const_pool, \
             tc.tile_pool(name="gpool", bufs=3) as gpool, \
             tc.tile_pool(name="small", bufs=2) as small, \
             tc.tile_pool(name="canv", bufs=2) as canv_pool, \
             tc.tile_pool(name="epi", bufs=2) as epi, \
             tc.tile_pool(name="argps", bufs=2, space="PSUM") as argps, \
             tc.tile_pool(name="canps", bufs=2, space="PSUM") as canps:

            qt = const_pool.tile([128, 2 * IMG], dt.bfloat16)
            nc.sync.dma_start(qt[:], q_in[:])
            outsb = const_pool.tile([128, 2 * NB], dt.float32)
            nc.vector.memset(outsb[:], 0.0)
            m100 = const_pool.tile([128, IMG], dt.float32)
            nc.vector.memset(m100[:], -100.0)
            mant_mask = const_pool.tile([128, 1], dt.int32)
            nc.vector.memset(mant_mask[:], 0x007FFFFF)
            one_bits = const_pool.tile([128, 1], dt.int32)
            nc.vector.memset(one_bits[:], 0x3F800000)

            # per-slot coefficient tiles: DMA rows 0-23, zero rows 24-127
            fts = []
            for n in range(NB):
                W = Ts[n] * 128
                ft = const_pool.tile([128, W], dt.bfloat16, name=f"ft{n}")
                for pb in (32, 64, 96):
                    nc.vector.memset(ft[pb:pb + 32, :], 0.0)
                nsplit = min(4, Ts[n])
                bounds = [round(i * Ts[n] / nsplit) * 128
                          for i in range(nsplit + 1)]
                for i in range(nsplit):
                    nc.sync.dma_start(ft[0:32, bounds[i]:bounds[i + 1]],
                                      f_ins[n][:, bounds[i]:bounds[i + 1]])
                fts.append(ft)

            canvases = []
            for n in range(NB):
                T = Ts[n]
                ft = fts[n]
                canvas_ps = canps.tile([128, IMG], dt.float32,
                                       name="canvas_ps")
                ch = 0
                while ch < T:
                    g = min(GRP, T - ch)
                    arg_ps = argps.tile([128, GRP * 2 * IMG], dt.float32,
                                        name="arg_ps")
                    for i in range(g):
                        nc.tensor.matmul(
                            arg_ps[:, i * 2 * IMG:(i + 1) * 2 * IMG],
                            ft[:, (ch + i) * 128:(ch + i + 1) * 128], qt[:],
                            start=True, stop=True)
                    gxy = gpool.tile([128, GRP * 2 * IMG], dt.bfloat16,
                                     name="gxy")
                    nc.scalar.activation(gxy[:, :g * 2 * IMG],
                                         arg_ps[:, :g * 2 * IMG], AF.Exp)
                    for i in range(g):
                        o = i * 2 * IMG
                        nc.tensor.matmul(
                            canvas_ps[:],
                            gxy[:, o:o + IMG], gxy[:, o + IMG:o + 2 * IMG],
                            start=(ch + i == 0), stop=(ch + i == T - 1))
                    ch += g

                canvas_sb = canv_pool.tile([128, IMG], dt.float32,
                                           name="canvas_sb", bufs=NB)
                nc.vector.tensor_copy(canvas_sb[:], canvas_ps[:])
                canvases.append(canvas_sb)

            # ---- epilogue: BCE terms (deferred; tanh phase then ln phase
            # so ACT table sets load at most once each)
            lines = []
            for n in range(NB):
                line = epi.tile([128, IMG], dt.float32, name="line", bufs=NB)
                nc.scalar.activation(line[:], canvases[n][:], AF.Tanh)
                lines.append(line)
            for n in range(NB):
                canvas_sb = canvases[n]
                line = lines[n]
                imgt = small.tile([128, IMG], dt.float32, name="imgt")
                nc.sync.dma_start(imgt[:], img_in[n])

                # The Ln LUT is inaccurate below ~1e-7, but line spans down to
                # 1e-38. Exact range reduction instead:
                #   ln(x) = ln(mant in [1,2)) + (bits - mant_bits) * ln2/2^23
                # (the int subtract isolates the exponent field exactly).
                xb = line[:].bitcast(dt.int32)
                mb = epi.tile([128, IMG], dt.int32, name="mb")
                nc.vector.tensor_scalar(mb[:], xb, mant_mask[:, 0:1],
                                        one_bits[:, 0:1],
                                        ALU.bitwise_and, ALU.bitwise_or)
                db = epi.tile([128, IMG], dt.int32, name="db")
                nc.vector.tensor_tensor(db[:], xb, mb[:], ALU.subtract)
                ef = epi.tile([128, IMG], dt.float32, name="ef")
                nc.vector.tensor_copy(ef[:], db[:])
                nc.vector.tensor_scalar(ef[:], ef[:],
                                        0.6931471805599453 / (1 << 23),
                                        None, ALU.mult)
                logp = epi.tile([128, IMG], dt.float32, name="logp")
                nc.scalar.activation(logp[:], mb[:].bitcast(dt.float32), AF.Ln)
                nc.vector.tensor_tensor(logp[:], logp[:], ef[:], ALU.add)
                maskt = epi.tile([128, IMG], dt.uint8, name="maskt")
                nc.vector.tensor_scalar(maskt[:], line[:], 1e-38, None,
                                        ALU.is_lt)
                nc.vector.copy_predicated(logp[:], maskt[:], m100[:])
                u = epi.tile([128, IMG], dt.float32, name="u")
                nc.vector.tensor_scalar(u[:], line[:], -1.0, 1.0,
                                        ALU.mult, ALU.add)
                nc.vector.tensor_scalar(u[:], u[:], ULP_BELOW_1, None,
                                        ALU.max)
                log1mp = epi.tile([128, IMG], dt.float32, name="log1mp")
                nc.scalar.activation(log1mp[:], u[:], AF.Ln)
                nc.vector.tensor_scalar(maskt[:], canvas_sb[:], TANH_SAT,
                                        None, ALU.is_ge)
                nc.vector.copy_predicated(log1mp[:], maskt[:], m100[:])
                diff = epi.tile([128, IMG], dt.float32, name="diff")
                nc.vector.tensor_tensor(diff[:], logp[:], log1mp[:],
                                        ALU.subtract)
                prod = epi.tile([128, IMG], dt.float32, name="prod")
                nc.vector.tensor_tensor(prod[:], imgt[:], diff[:], ALU.mult)
                tot = epi.tile([128, IMG], dt.float32, name="tot")
                nc.vector.tensor_tensor(tot[:], prod[:], log1mp[:], ALU.add)
                nc.vector.tensor_reduce(outsb[:, n:n + 1], tot[:],
                                        mybir.AxisListType.X, ALU.add)

                # ---- distance term
                ta = small.tile([P - 1, 2], dt.float32, name="ta")
                tb = small.tile([P - 1, 2], dt.float32, name="tb")
                nc.sync.dma_start(ta[:], ptsa_in[n])
                nc.sync.dma_start(tb[:], ptsb_in[n])
                dxy = epi.tile([P - 1, 2], dt.float32, name="dxy")
                nc.vector.tensor_tensor(dxy[:], tb[:], ta[:], ALU.subtract)
                nc.vector.tensor_tensor(dxy[:], dxy[:], dxy[:], ALU.mult)
                segsq = epi.tile([P - 1, 1], dt.float32, name="segsq")
                nc.vector.tensor_reduce(segsq[:], dxy[:],
                                        mybir.AxisListType.X, ALU.add)
                dx = epi.tile([P - 1, 1], dt.float32, name="dx")
                nc.vector.tensor_scalar(dx[:], segsq[:], -X0, None, ALU.add)
                poly = epi.tile([P - 1, 1], dt.float32, name="poly")
                nc.vector.tensor_scalar(poly[:], dx[:], C3, C2,
                                        ALU.mult, ALU.add)
                nc.vector.tensor_tensor(poly[:], poly[:], dx[:], ALU.mult)
                nc.vector.tensor_scalar(poly[:], poly[:], C1, None, ALU.add)
                nc.vector.tensor_tensor(poly[:], poly[:], dx[:], ALU.mult)
                nc.vector.tensor_scalar(outsb[:P - 1, NB + n:NB + n + 1],
                                        poly[:], C0, None, ALU.add)

            nc.sync.dma_start(out[:], outsb[:])
    nc.compile()
    return nc


_NC_CACHE = {}


def _get_nc(Ts):
    if Ts not in _NC_CACHE:
        _NC_CACHE[Ts] = _build_nc(Ts)
    return _NC_CACHE[Ts]


def make_in_maps(points, img, Ts, core_batches, samples):
    points = np.asarray(points, np.float32)
    img = np.asarray(img, np.float32)
    q24 = _build_q24()
    in_maps = []
    for c in range(NCORES):
        bl = core_batches[c]
        pts = points[bl]
        im = {f"fs{n}": np.ascontiguousarray(
                  _build_f24(*samples[bl[n]], Ts[n] * 128))
              for n in range(NB)}
        im.update({
            "q24": q24,
            "img": np.ascontiguousarray(img[bl]),
            "ptsa": np.ascontiguousarray(pts[:, :P - 1, 0:2]),
            "ptsb": np.ascontiguousarray(pts[:, 1:, 0:2]),
        })
        in_maps.append(im)
    return in_maps


def combine_outputs(results):
    bce_tot = 0.0
    dist_tot = 0.0
    for r in results:
        o = np.asarray(r["out"], np.float64)
        bce_tot += o[:, :NB].sum()
        dist_tot += o[:P - 1, NB:].sum()
    return np.float32((dist_tot - bce_tot) / N)


def kernel(points, img, _trace=False, _trace_kwargs=None):
    Ts, core_batches, samples = _schedule(points)
    nc = _get_nc(Ts)
    in_maps = make_in_maps(points, img, Ts, core_batches, samples)
    kw = {}
    if _trace:
        kw.update(trace=True, trace_cores=[0])
        if _trace_kwargs:
            kw.update(_trace_kwargs)
    res = run_bass_kernel_spmd(nc, in_maps, core_ids=list(range(NCORES)), **kw)
    out = combine_outputs(res.results)
    if _trace:
        return out, res
    return out


# revision 6
# speedup vs baseline: 3.1821x; 1.2764x over previous
"""Trainium2 Bass kernel for nn_Discriminator (histogram_binning / ridge).

Math (reference):
  For each batch n (N=32): interpolate P=128 points into M=(P-1)*181=22987
  line points (x,y,w); splat Gaussians g_x[m,s]=exp(-(x_m-s)^2/(2 w_m)),
  g_y[m,t]; canvas = g_x^T @ g_y  [128,128]; line = tanh(canvas);
  loss = sum(BCE(line, img))/N + sum(poly_sqrt(seg_len^2))/N.

Key optimization (adaptive segment resampling):
  The reference samples every segment at 181 points (one per ~0.37 px),
  but the splatted Gaussian has sigma = sqrt(w) >= 0.71 px; a trapezoid
  rule at spacing h needs only h <= ALPHA*sqrt(w) for aliasing error
  2*exp(-2*pi^2*w/h^2) (Poisson summation).  Per segment we place
  nseg+1 = ceil(len/(ALPHA*sqrt(w_min)))+1 trapezoid nodes spanning
  t in [0, 180/181]; interior amplitude r = 180/nseg, endpoint
  amplitude (r+1)/2 reproduces the reference's 181-term sum up to
  ~1e-3 relative.  This cuts M ~6.7x (23040 -> ~3400 per batch).
  Amplitudes fold into the Gaussian offset: c0x += ln(amp).

Device strategy (data-parallel over N, 4 batches per core, 8 cores;
batches assigned to the 4 slots by descending chunk count so every
slot's chunk count T_n is tight across cores):
  The Gaussian exponent arg[m,s] = c2[m]*s'^2 + c1[m]*s' + c0[m] (s'=s-64)
  is computed on the TensorEngine as a K=24 bf16 matmul (zero-padded to
  K=128): basis rows (s'^2 split into two exact bf16 rows, s', 1), each
  coefficient split into 3 bf16 levels (~25-bit mantissa).  A block-
  diagonal basis computes x-arg and y-arg in one matmul
  ([128,128m] lhsT x [128,256] rhs).  ScalarE applies Exp (PSUM->SBUF,
  bf16 out), and the canvas accumulates T_n chunk matmuls (K=128, bf16)
  in PSUM.  tanh/log/BCE epilogue per batch; final sums on host.
  Dead padding points use c0 = -200 so exp underflows to exactly 0.
"""
import sys
import types
import numpy as np
import ml_dtypes

# ---------------------------------------------------------------- constants
IMG = 128          # image size S
P = 128            # points per batch
N = 32             # batch
CMP = int(IMG * np.sqrt(2))            # 181
NCORES = 8
NB = N // NCORES                       # 4 batches per core
GRP = 6                                # arg chunks per Exp instruction
CENTER = 64.0
ALPHA = 3.0                            # resampling spacing, in sigmas

_d = np.arange(-IMG + 1, IMG)
X0 = float((_d ** 2 + (_d ** 2).T).mean().astype(np.float32))
C0 = float(X0 ** 0.5)
C1 = float(X0 ** (-0.5) / 2.0)
C2 = float(-(X0 ** (-1.5) / 8.0))
C3 = float(X0 ** (-2.5) / 16.0)

_BF = ml_dtypes.bfloat16

# XLA:CPU f32 tanh returns exactly 1.0 for x >= this (empirical, bit-exact);
# the reference's clip(log(1-line), -100) then yields -100 on those pixels.
TANH_SAT = float(np.uint32(1090516548).view(np.float32))  # 7.9988117
ULP_BELOW_1 = 5.960464477539063e-08  # 1 - nextafter(1, 0) in f32


def _install_ntff_hook():
    """bass_utils wants antenv.axon_hooks for trace=True under axon; the image
    lacks it. Provide it, backed by the ctypes shim in trn_agent_boot."""
    if 'antenv.axon_hooks' in sys.modules:
        return
    mod = types.ModuleType('antenv.axon_hooks')
    _h = [None]
    mod.set_axon_ntff_profile_hook = lambda h: _h.__setitem__(0, h)
    mod.get_axon_ntff_profile_hook = lambda: _h[0]
    sys.modules['antenv.axon_hooks'] = mod
    try:
        from trn_agent_boot.trn_boot import _ntff_profile_via_ctypes
        mod.set_axon_ntff_profile_hook(
            _ntff_profile_via_ctypes('/opt/axon/libaxon_pjrt.so'))
    except Exception:
        pass


_install_ntff_hook()

import concourse.bass as bass          # noqa: E402
import concourse.tile as tile          # noqa: E402
from concourse import bacc, mybir      # noqa: E402
from concourse.bass_utils import run_bass_kernel_spmd  # noqa: E402

dt = mybir.dt
AF = mybir.ActivationFunctionType
ALU = mybir.AluOpType


# ---------------------------------------------------------------- host prep
def _bf16_split3(x):
    h = x.astype(_BF).astype(np.float64)
    m = (x - h).astype(_BF).astype(np.float64)
    l = (x - h - m).astype(_BF).astype(np.float64)
    return h, m, l


def _build_q24():
    """Block-diagonal exact bf16 basis, zero-padded to K=128 rows (the PE's
    HAM clock-gate only counts full-K matmuls as activity)."""
    sprime = np.arange(IMG, dtype=np.float64) - CENTER
    s2 = sprime ** 2
    s2h = s2.astype(_BF).astype(np.float64)
    s2l = s2 - s2h
    qrows = [s2h, s2l, sprime, np.ones(IMG)]
    q = np.zeros((128, 2 * IMG))
    for base, off in ((0, 0), (12, IMG)):
        for lvl in range(3):
            for j in range(4):
                q[base + lvl * 4 + j, off:off + IMG] = qrows[j]
    return q.astype(_BF)


def _resample_batch(pts):
    """pts [P,3] f64 -> (xs, ys, ws, amps) trapezoid-node resampling of the
    reference's per-segment 181-point splat."""
    a = pts[:-1]                                   # [P-1, 3]
    b = pts[1:]
    tJ = (CMP - 1) / CMP
    e = a + (b - a) * tJ                           # last fine sample per seg
    seglen = np.sqrt(((b[:, :2] - a[:, :2]) ** 2).sum(-1)) * tJ
    wmin = np.minimum(a[:, 2], e[:, 2])
    nseg = np.clip(np.ceil(seglen / (ALPHA * np.sqrt(wmin))), 1,
                   CMP - 1).astype(int)
    xs_l, ys_l, ws_l, am_l = [], [], [], []
    for s in range(P - 1):
        ns = nseg[s]
        ti = np.arange(ns + 1) * (tJ / ns)
        r = (CMP - 1) / ns
        amp = np.full(ns + 1, r)
        amp[0] = amp[-1] = (r + 1) / 2
        xs_l.append(a[s, 0] + (b[s, 0] - a[s, 0]) * ti)
        ys_l.append(a[s, 1] + (b[s, 1] - a[s, 1]) * ti)
        ws_l.append(a[s, 2] + (b[s, 2] - a[s, 2]) * ti)
        am_l.append(amp)
    return (np.concatenate(xs_l), np.concatenate(ys_l),
            np.concatenate(ws_l), np.concatenate(am_l))


def _build_f24(xs, ys, ws, amps, width):
    """samples -> F [32, width] bf16 coefficient rows (padded with dead
    points whose exp underflows to exactly 0)."""
    m = len(xs)
    x = xs - CENTER
    y = ys - CENTER
    invw = 1.0 / ws
    c2 = -0.5 * invw
    c1x = x * invw
    c0x = -0.5 * x * x * invw + np.log(amps)
    c1y = y * invw
    c0y = -0.5 * y * y * invw

    F = np.zeros((32, width))
    for base, c1_, c0_ in ((0, c1x, c0x), (12, c1y, c0y)):
        splits = [_bf16_split3(c2), _bf16_split3(c2),
                  _bf16_split3(c1_), _bf16_split3(c0_)]
        for lvl in range(3):
            for j in range(4):
                F[base + lvl * 4 + j, :m] = splits[j][lvl]
    # dead padding: arg_x = arg_y = -200 -> exp == 0 exactly (f32 underflow)
    F[3, m:] = -200.0
    F[15, m:] = -200.0
    return F.astype(_BF)


def _schedule(points):
    """Resample all batches, assign them to (core, slot) so each slot's
    chunk count is tight. Returns per-slot widths Ts and per-core data."""
    points = np.asarray(points, np.float64)
    samples = [_resample_batch(points[b]) for b in range(N)]
    chunks = np.array([(len(s[0]) + 127) // 128 for s in samples])
    order = np.argsort(-chunks, kind='stable')
    Ts = []
    core_batches = [[] for _ in range(NCORES)]
    for n in range(NB):
        grp = order[n * NCORES:(n + 1) * NCORES]
        Ts.append(int(chunks[grp].max()))
        for c in range(NCORES):
            core_batches[c].append(int(grp[c]))
    return tuple(Ts), core_batches, samples


# ---------------------------------------------------------------- device
def _build_nc(Ts):
    nc = bacc.Bacc("TRN2", target_bir_lowering=False, debug=False,
                   enable_asserts=False, num_devices=NCORES)
    f_ins = [nc.dram_tensor(f"fs{n}", [32, Ts[n] * 128], dt.bfloat16,
                            kind="ExternalInput").ap()
             for n in range(NB)]
    q_in = nc.dram_tensor("q24", [128, 2 * IMG], dt.bfloat16,
                          kind="ExternalInput").ap()
    img_in = nc.dram_tensor("img", [NB, IMG, IMG], dt.float32,
                            kind="ExternalInput").ap()
    ptsa_in = nc.dram_tensor("ptsa", [NB, P - 1, 2], dt.float32,
                             kind="ExternalInput").ap()
    ptsb_in = nc.dram_tensor("ptsb", [NB, P - 1, 2], dt.float32,
                             kind="ExternalInput").ap()
    out = nc.dram_tensor("out", [128, 2 * NB], dt.float32,
                         kind="ExternalOutput").ap()
    W4 = NB * IMG

    with tile.TileContext(nc) as tc:
        with tc.tile_pool(name="const", bufs=1) as const_pool, \
             tc.tile_pool(name="gpool", bufs=3) as gpool, \
             tc.tile_pool(name="small", bufs=2) as small, \
             tc.tile_pool(name="epi", bufs=2) as epi, \
             tc.tile_pool(name="argps", bufs=2, space="PSUM") as argps, \
             tc.tile_pool(name="canps", bufs=2, space="PSUM") as canps:

            qt = const_pool.tile([128, 2 * IMG], dt.bfloat16)
            nc.sync.dma_start(qt[:], q_in[:])
            outsb = const_pool.tile([128, 2 * NB], dt.float32)
            nc.vector.memset(outsb[:], 0.0)
            m100 = const_pool.tile([128, W4], dt.float32)
            nc.vector.memset(m100[:], -100.0)
            mant_mask = const_pool.tile([128, 1], dt.int32)
            nc.vector.memset(mant_mask[:], 0x007FFFFF)
            one_bits = const_pool.tile([128, 1], dt.int32)
            nc.vector.memset(one_bits[:], 0x3F800000)

            # prefetch the Exp table set while DMAs run
            warm = const_pool.tile([128, 1], dt.float32)
            nc.scalar.activation(warm[:], m100[:, 0:1], AF.Exp)

            # per-slot coefficient tiles (24 used rows + 8 zero rows; the
            # arg matmul contracts K=32 so no high-partition zeroing needed)
            fts = []
            for n in range(NB):
                W = Ts[n] * 128
                ft = const_pool.tile([32, W], dt.bfloat16, name=f"ft{n}")
                nsplit = min(4, Ts[n])
                bounds = [round(i * Ts[n] / nsplit) * 128
                          for i in range(nsplit + 1)]
                for i in range(nsplit):
                    nc.sync.dma_start(ft[:, bounds[i]:bounds[i + 1]],
                                      f_ins[n][:, bounds[i]:bounds[i + 1]])
                fts.append(ft)

            imgt = const_pool.tile([128, W4], dt.float32)
            for n in range(NB):
                nc.sync.dma_start(imgt[:, n * IMG:(n + 1) * IMG], img_in[n])

            # ---- distance term (independent; fills DVE during startup)
            for n in range(NB):
                ta = small.tile([P - 1, 2], dt.float32, name="ta")
                tb = small.tile([P - 1, 2], dt.float32, name="tb")
                nc.sync.dma_start(ta[:], ptsa_in[n])
                nc.sync.dma_start(tb[:], ptsb_in[n])
                dxy = epi.tile([P - 1, 2], dt.float32, name="dxy")
                nc.vector.tensor_tensor(dxy[:], tb[:], ta[:], ALU.subtract)
                nc.vector.tensor_tensor(dxy[:], dxy[:], dxy[:], ALU.mult)
                segsq = epi.tile([P - 1, 1], dt.float32, name="segsq")
                nc.vector.tensor_reduce(segsq[:], dxy[:],
                                        mybir.AxisListType.X, ALU.add)
                dx = epi.tile([P - 1, 1], dt.float32, name="dx")
                nc.vector.tensor_scalar(dx[:], segsq[:], -X0, None, ALU.add)
                poly = epi.tile([P - 1, 1], dt.float32, name="poly")
                nc.vector.tensor_scalar(poly[:], dx[:], C3, C2,
                                        ALU.mult, ALU.add)
                nc.vector.tensor_tensor(poly[:], poly[:], dx[:], ALU.mult)
                nc.vector.tensor_scalar(poly[:], poly[:], C1, None, ALU.add)
                nc.vector.tensor_tensor(poly[:], poly[:], dx[:], ALU.mult)
                nc.vector.tensor_scalar(outsb[:P - 1, NB + n:NB + n + 1],
                                        poly[:], C0, None, ALU.add)

            # persistent epilogue accumulators, written per batch slice
            mbA = const_pool.tile([128, W4], dt.int32)
            efA = const_pool.tile([128, W4], dt.float32)
            upA = const_pool.tile([128, W4], dt.float32)
            snmA = const_pool.tile([128, W4], dt.uint8)
            satA = const_pool.tile([128, W4], dt.uint8)

            for n in range(NB):
                T = Ts[n]
                ft = fts[n]
                canvas_ps = canps.tile([128, IMG], dt.float32,
                                       name="canvas_ps")
                ch = 0
                while ch < T:
                    g = min(GRP, T - ch)
                    arg_ps = argps.tile([128, GRP * 2 * IMG], dt.float32,
                                        name="arg_ps")
                    for i in range(g):
                        nc.tensor.matmul(
                            arg_ps[:, i * 2 * IMG:(i + 1) * 2 * IMG],
                            ft[:, (ch + i) * 128:(ch + i + 1) * 128],
                            qt[0:32, :], start=True, stop=True)
                    gxy = gpool.tile([128, GRP * 2 * IMG], dt.bfloat16,
                                     name="gxy")
                    nc.scalar.activation(gxy[:, :g * 2 * IMG],
                                         arg_ps[:, :g * 2 * IMG], AF.Exp)
                    for i in range(g):
                        o = i * 2 * IMG
                        nc.tensor.matmul(
                            canvas_ps[:],
                            gxy[:, o:o + IMG], gxy[:, o + IMG:o + 2 * IMG],
                            start=(ch + i == 0), stop=(ch + i == T - 1))
                    ch += g

                # per-batch pre-Ln epilogue (tanh shares the Exp table set;
                # DVE work hides under the next batch's main loop)
                sl = slice(n * IMG, (n + 1) * IMG)
                line = epi.tile([128, IMG], dt.float32, name="line")
                nc.scalar.activation(line[:], canvas_ps[:], AF.Tanh)
                nc.vector.tensor_scalar(satA[:, sl], canvas_ps[:], TANH_SAT,
                                        None, ALU.is_ge)
                xb = line[:].bitcast(dt.int32)
                nc.vector.tensor_scalar(mbA[:, sl], xb, mant_mask[:, 0:1],
                                        one_bits[:, 0:1],
                                        ALU.bitwise_and, ALU.bitwise_or)
                db = epi.tile([128, IMG], dt.int32, name="db")
                nc.vector.tensor_tensor(db[:], xb, mbA[:, sl], ALU.subtract)
                nc.vector.tensor_copy(efA[:, sl], db[:])
                nc.vector.tensor_scalar(efA[:, sl], efA[:, sl],
                                        0.6931471805599453 / (1 << 23),
                                        None, ALU.mult)
                nc.vector.tensor_scalar(snmA[:, sl], line[:], 1e-38, None,
                                        ALU.is_lt)
                nc.vector.tensor_scalar(upA[:, sl], line[:], -1.0, 1.0,
                                        ALU.mult, ALU.add)
                nc.vector.tensor_scalar(upA[:, sl], upA[:, sl], ULP_BELOW_1,
                                        None, ALU.max)

            # ---- deferred Ln phase (single table switch), batched over NB
            lnm = const_pool.tile([128, W4], dt.float32)
            lgm = const_pool.tile([128, W4], dt.float32)
            nc.scalar.activation(lnm[:], mbA[:].bitcast(dt.float32), AF.Ln)
            nc.scalar.activation(lgm[:], upA[:], AF.Ln)
            nc.vector.tensor_tensor(lnm[:], lnm[:], efA[:], ALU.add)
            nc.vector.copy_predicated(lnm[:], snmA[:], m100[:])
            nc.vector.copy_predicated(lgm[:], satA[:], m100[:])
            nc.vector.tensor_tensor(lnm[:], lnm[:], lgm[:], ALU.subtract)
            nc.vector.tensor_tensor(lnm[:], imgt[:], lnm[:], ALU.mult)
            nc.vector.tensor_tensor(lnm[:], lnm[:], lgm[:], ALU.add)
            nc.vector.tensor_reduce(outsb[:, 0:1], lnm[:],
                                    mybir.AxisListType.X, ALU.add)

            nc.sync.dma_start(out[:], outsb[:])
    nc.compile()
    return nc


_NC_CACHE = {}


def _get_nc(Ts):
    if Ts not in _NC_CACHE:
        _NC_CACHE[Ts] = _build_nc(Ts)
    return _NC_CACHE[Ts]


def make_in_maps(points, img, Ts, core_batches, samples):
    points = np.asarray(points, np.float32)
    img = np.asarray(img, np.float32)
    q24 = _build_q24()
    in_maps = []
    for c in range(NCORES):
        bl = core_batches[c]
        pts = points[bl]
        im = {f"fs{n}": np.ascontiguousarray(
                  _build_f24(*samples[bl[n]], Ts[n] * 128))
              for n in range(NB)}
        im.update({
            "q24": q24,
            "img": np.ascontiguousarray(img[bl]),
            "ptsa": np.ascontiguousarray(pts[:, :P - 1, 0:2]),
            "ptsb": np.ascontiguousarray(pts[:, 1:, 0:2]),
        })
        in_maps.append(im)
    return in_maps


def combine_outputs(results):
    bce_tot = 0.0
    dist_tot = 0.0
    for r in results:
        o = np.asarray(r["out"], np.float64)
        bce_tot += o[:, 0].sum()
        dist_tot += o[:P - 1, NB:].sum()
    return np.float32((dist_tot - bce_tot) / N)


def kernel(points, img, _trace=False, _trace_kwargs=None):
    Ts, core_batches, samples = _schedule(points)
    nc = _get_nc(Ts)
    in_maps = make_in_maps(points, img, Ts, core_batches, samples)
    kw = {}
    if _trace:
        kw.update(trace=True, trace_cores=[0])
        if _trace_kwargs:
            kw.update(_trace_kwargs)
    res = run_bass_kernel_spmd(nc, in_maps, core_ids=list(range(NCORES)), **kw)
    out = combine_outputs(res.results)
    if _trace:
        return out, res
    return out


# revision 10
# speedup vs baseline: 3.2154x; 1.0105x over previous
"""Trainium2 Bass kernel for nn_Discriminator (histogram_binning / ridge).

Math (reference):
  For each batch n (N=32): interpolate P=128 points into M=(P-1)*181=22987
  line points (x,y,w); splat Gaussians g_x[m,s]=exp(-(x_m-s)^2/(2 w_m)),
  g_y[m,t]; canvas = g_x^T @ g_y  [128,128]; line = tanh(canvas);
  loss = sum(BCE(line, img))/N + sum(poly_sqrt(seg_len^2))/N.

Key optimization (adaptive segment resampling):
  The reference samples every segment at 181 points (one per ~0.37 px),
  but the splatted Gaussian has sigma = sqrt(w) >= 0.71 px; a trapezoid
  rule at spacing h needs only h <= ALPHA*sqrt(w) for aliasing error
  2*exp(-2*pi^2*w/h^2) (Poisson summation).  Per segment we place
  nseg+1 = ceil(len/(ALPHA*sqrt(w_min)))+1 trapezoid nodes spanning
  t in [0, 180/181]; interior amplitude r = 180/nseg, endpoint
  amplitude (r+1)/2 reproduces the reference's 181-term sum up to
  ~1e-3 relative.  This cuts M ~6.7x (23040 -> ~3400 per batch).
  Amplitudes fold into the Gaussian offset: c0x += ln(amp).

Device strategy (data-parallel over N, 4 batches per core, 8 cores;
batches assigned to the 4 slots by descending chunk count so every
slot's chunk count T_n is tight across cores):
  The Gaussian exponent arg[m,s] = c2[m]*s'^2 + c1[m]*s' + c0[m] (s'=s-64)
  is computed on the TensorEngine as a K=24 bf16 matmul (zero-padded to
  K=128): basis rows (s'^2 split into two exact bf16 rows, s', 1), each
  coefficient split into 3 bf16 levels (~25-bit mantissa).  A block-
  diagonal basis computes x-arg and y-arg in one matmul
  ([128,128m] lhsT x [128,256] rhs).  ScalarE applies Exp (PSUM->SBUF,
  bf16 out), and the canvas accumulates T_n chunk matmuls (K=128, bf16)
  in PSUM.  tanh/log/BCE epilogue per batch; final sums on host.
  Dead padding points use c0 = -200 so exp underflows to exactly 0.
"""
import sys
import types
import numpy as np
import ml_dtypes

# ---------------------------------------------------------------- constants
IMG = 128          # image size S
P = 128            # points per batch
N = 32             # batch
CMP = int(IMG * np.sqrt(2))            # 181
NCORES = 8
NB = N // NCORES                       # 4 batches per core
GRP = 6                                # arg chunks per Exp instruction
CENTER = 64.0
ALPHA = 3.5                            # resampling spacing, in sigmas

_d = np.arange(-IMG + 1, IMG)
X0 = float((_d ** 2 + (_d ** 2).T).mean().astype(np.float32))
C0 = float(X0 ** 0.5)
C1 = float(X0 ** (-0.5) / 2.0)
C2 = float(-(X0 ** (-1.5) / 8.0))
C3 = float(X0 ** (-2.5) / 16.0)

_BF = ml_dtypes.bfloat16

# XLA:CPU f32 tanh returns exactly 1.0 for x >= this (empirical, bit-exact);
# the reference's clip(log(1-line), -100) then yields -100 on those pixels.
TANH_SAT = float(np.uint32(1090516548).view(np.float32))  # 7.9988117
ULP_BELOW_1 = 5.960464477539063e-08  # 1 - nextafter(1, 0) in f32


def _install_ntff_hook():
    """bass_utils wants antenv.axon_hooks for trace=True under axon; the image
    lacks it. Provide it, backed by the ctypes shim in trn_agent_boot."""
    if 'antenv.axon_hooks' in sys.modules:
        return
    mod = types.ModuleType('antenv.axon_hooks')
    _h = [None]
    mod.set_axon_ntff_profile_hook = lambda h: _h.__setitem__(0, h)
    mod.get_axon_ntff_profile_hook = lambda: _h[0]
    sys.modules['antenv.axon_hooks'] = mod
    try:
        from trn_agent_boot.trn_boot import _ntff_profile_via_ctypes
        mod.set_axon_ntff_profile_hook(
            _ntff_profile_via_ctypes('/opt/axon/libaxon_pjrt.so'))
    except Exception:
        pass


_install_ntff_hook()

import concourse.bass as bass          # noqa: E402
import concourse.tile as tile          # noqa: E402
from concourse import bacc, mybir      # noqa: E402
from concourse.bass_utils import run_bass_kernel_spmd  # noqa: E402

dt = mybir.dt
AF = mybir.ActivationFunctionType
ALU = mybir.AluOpType


# ---------------------------------------------------------------- host prep
def _bf16_split3(x):
    h = x.astype(_BF).astype(np.float64)
    m = (x - h).astype(_BF).astype(np.float64)
    l = (x - h - m).astype(_BF).astype(np.float64)
    return h, m, l


def _build_q24():
    """Block-diagonal exact bf16 basis, zero-padded to K=128 rows (the PE's
    HAM clock-gate only counts full-K matmuls as activity)."""
    sprime = np.arange(IMG, dtype=np.float64) - CENTER
    s2 = sprime ** 2
    s2h = s2.astype(_BF).astype(np.float64)
    s2l = s2 - s2h
    qrows = [s2h, s2l, sprime, np.ones(IMG)]
    q = np.zeros((128, 2 * IMG))
    for base, off in ((0, 0), (12, IMG)):
        for lvl in range(3):
            for j in range(4):
                q[base + lvl * 4 + j, off:off + IMG] = qrows[j]
    return q.astype(_BF)


def _resample_batch(pts):
    """pts [P,3] f64 -> (xs, ys, ws, amps) trapezoid-node resampling of the
    reference's per-segment 181-point splat."""
    a = pts[:-1]                                   # [P-1, 3]
    b = pts[1:]
    tJ = (CMP - 1) / CMP
    e = a + (b - a) * tJ                           # last fine sample per seg
    seglen = np.sqrt(((b[:, :2] - a[:, :2]) ** 2).sum(-1)) * tJ
    wmin = np.minimum(a[:, 2], e[:, 2])
    nseg = np.clip(np.ceil(seglen / (ALPHA * np.sqrt(wmin))), 1,
                   CMP - 1).astype(int)
    xs_l, ys_l, ws_l, am_l = [], [], [], []
    for s in range(P - 1):
        ns = nseg[s]
        ti = np.arange(ns + 1) * (tJ / ns)
        r = (CMP - 1) / ns
        amp = np.full(ns + 1, r)
        amp[0] = amp[-1] = (r + 1) / 2
        xs_l.append(a[s, 0] + (b[s, 0] - a[s, 0]) * ti)
        ys_l.append(a[s, 1] + (b[s, 1] - a[s, 1]) * ti)
        ws_l.append(a[s, 2] + (b[s, 2] - a[s, 2]) * ti)
        am_l.append(amp)
    return (np.concatenate(xs_l), np.concatenate(ys_l),
            np.concatenate(ws_l), np.concatenate(am_l))


def _build_f24(xs, ys, ws, amps, width):
    """samples -> F [32, width] bf16 coefficient rows (padded with dead
    points whose exp underflows to exactly 0)."""
    m = len(xs)
    x = xs - CENTER
    y = ys - CENTER
    invw = 1.0 / ws
    c2 = -0.5 * invw
    c1x = x * invw
    c0x = -0.5 * x * x * invw + np.log(amps)
    c1y = y * invw
    c0y = -0.5 * y * y * invw

    F = np.zeros((32, width))
    for base, c1_, c0_ in ((0, c1x, c0x), (12, c1y, c0y)):
        splits = [_bf16_split3(c2), _bf16_split3(c2),
                  _bf16_split3(c1_), _bf16_split3(c0_)]
        for lvl in range(3):
            for j in range(4):
                F[base + lvl * 4 + j, :m] = splits[j][lvl]
    # dead padding: arg_x = arg_y = -200 -> exp == 0 exactly (f32 underflow)
    F[3, m:] = -200.0
    F[15, m:] = -200.0
    return F.astype(_BF)


def _schedule(points):
    """Resample all batches, assign them to (core, slot) so each slot's
    chunk count is tight. Returns per-slot widths Ts and per-core data."""
    points = np.asarray(points, np.float64)
    samples = [_resample_batch(points[b]) for b in range(N)]
    chunks = np.array([(len(s[0]) + 127) // 128 for s in samples])
    order = np.argsort(-chunks, kind='stable')
    Ts = []
    core_batches = [[] for _ in range(NCORES)]
    for n in range(NB):
        grp = order[n * NCORES:(n + 1) * NCORES]
        Ts.append(int(chunks[grp].max()))
        for c in range(NCORES):
            core_batches[c].append(int(grp[c]))
    return tuple(Ts), core_batches, samples


# ---------------------------------------------------------------- device
def _build_nc(Ts):
    nc = bacc.Bacc("TRN2", target_bir_lowering=False, debug=False,
                   enable_asserts=False, num_devices=NCORES)
    f_ins = [nc.dram_tensor(f"fs{n}", [32, Ts[n] * 128], dt.bfloat16,
                            kind="ExternalInput").ap()
             for n in range(NB)]
    z_in = nc.dram_tensor("zros", [96, Ts[0] * 128], dt.bfloat16,
                          kind="ExternalInput").ap()
    q_in = nc.dram_tensor("q24", [128, 2 * IMG], dt.bfloat16,
                          kind="ExternalInput").ap()
    img_in = nc.dram_tensor("img", [NB, IMG, IMG], dt.float32,
                            kind="ExternalInput").ap()
    imgm_in = nc.dram_tensor("imgm", [NB, IMG, IMG], dt.float32,
                             kind="ExternalInput").ap()
    ptsa_in = nc.dram_tensor("ptsa", [NB, P - 1, 2], dt.float32,
                             kind="ExternalInput").ap()
    ptsb_in = nc.dram_tensor("ptsb", [NB, P - 1, 2], dt.float32,
                             kind="ExternalInput").ap()
    out = nc.dram_tensor("out", [128, 2 * NB], dt.float32,
                         kind="ExternalOutput").ap()
    W4 = NB * IMG

    with tile.TileContext(nc) as tc:
        with tc.tile_pool(name="const", bufs=1) as const_pool, \
             tc.tile_pool(name="fpool", bufs=2) as fpool, \
             tc.tile_pool(name="gpool", bufs=3) as gpool, \
             tc.tile_pool(name="small", bufs=2) as small, \
             tc.tile_pool(name="epi", bufs=2) as epi, \
             tc.tile_pool(name="argps", bufs=2, space="PSUM") as argps, \
             tc.tile_pool(name="canps", bufs=2, space="PSUM") as canps:

            qt = const_pool.tile([128, 2 * IMG], dt.bfloat16)
            nc.sync.dma_start(qt[:], q_in[:])
            outsb = const_pool.tile([128, 2 * NB], dt.float32)
            nc.vector.memset(outsb[:], 0.0)
            m100 = const_pool.tile([128, W4], dt.float32)
            nc.vector.memset(m100[:], -100.0)
            mant_mask = const_pool.tile([128, 1], dt.int32)
            nc.vector.memset(mant_mask[:], 0x007FFFFF)
            one_bits = const_pool.tile([128, 1], dt.int32)
            nc.vector.memset(one_bits[:], 0x3F800000)

            # dep-free warmups: preload the Exp table set and warm the PE
            # HAM clock gate while input DMAs are in flight (inputs are
            # uninitialized garbage; nothing reads the outputs)
            wsrc = const_pool.tile([128, 1], dt.float32, name="wsrc")
            nc.vector.memset(wsrc[:], 0.0)
            wdst = const_pool.tile([128, 1], dt.float32, name="wdst")
            nc.scalar.activation(wdst[:], wsrc[:], AF.Exp)
            wbf = const_pool.tile([128, 128], dt.bfloat16, name="wbf")
            nc.vector.memset(wbf[:], 1.0)
            wp = canps.tile([128, IMG], dt.float32, name="canvas_ps")
            for _ in range(25):
                nc.tensor.matmul(wp[:], wbf[:], wbf[:],
                                 start=True, stop=True)

            imgt = const_pool.tile([128, W4], dt.float32)
            imgmt = const_pool.tile([128, W4], dt.float32)
            for n in range(NB):
                nc.sync.dma_start(imgt[:, n * IMG:(n + 1) * IMG], img_in[n])
                nc.sync.dma_start(imgmt[:, n * IMG:(n + 1) * IMG],
                                  imgm_in[n])

            # ---- distance term (independent; fills DVE during startup)
            for n in range(NB):
                ta = small.tile([P - 1, 2], dt.float32, name="ta")
                tb = small.tile([P - 1, 2], dt.float32, name="tb")
                nc.sync.dma_start(ta[:], ptsa_in[n])
                nc.sync.dma_start(tb[:], ptsb_in[n])
                dxy = epi.tile([P - 1, 2], dt.float32, name="dxy")
                nc.vector.tensor_tensor(dxy[:], tb[:], ta[:], ALU.subtract)
                nc.vector.tensor_tensor(dxy[:], dxy[:], dxy[:], ALU.mult)
                segsq = epi.tile([P - 1, 1], dt.float32, name="segsq")
                nc.vector.tensor_reduce(segsq[:], dxy[:],
                                        mybir.AxisListType.X, ALU.add)
                dx = epi.tile([P - 1, 1], dt.float32, name="dx")
                nc.vector.tensor_scalar(dx[:], segsq[:], -X0, None, ALU.add)
                poly = epi.tile([P - 1, 1], dt.float32, name="poly")
                nc.vector.tensor_scalar(poly[:], dx[:], C3, C2,
                                        ALU.mult, ALU.add)
                nc.vector.tensor_tensor(poly[:], poly[:], dx[:], ALU.mult)
                nc.vector.tensor_scalar(poly[:], poly[:], C1, None, ALU.add)
                nc.vector.tensor_tensor(poly[:], poly[:], dx[:], ALU.mult)
                nc.vector.tensor_scalar(outsb[:P - 1, NB + n:NB + n + 1],
                                        poly[:], C0, None, ALU.add)

            # persistent epilogue accumulators, written per batch slice
            mbA = const_pool.tile([128, W4], dt.int32)
            efA = const_pool.tile([128, W4], dt.float32)
            upA = const_pool.tile([128, W4], dt.float32)
            snmA = const_pool.tile([128, W4], dt.uint8)
            satA = const_pool.tile([128, W4], dt.uint8)

            def emit_epilogue(n, canvas_ps):
                # per-batch pre-Ln epilogue (tanh shares the Exp table set;
                # DVE work hides under the next batch's main loop)
                sl = slice(n * IMG, (n + 1) * IMG)
                line = epi.tile([128, IMG], dt.float32, name="line")
                nc.scalar.activation(line[:], canvas_ps[:], AF.Tanh)
                nc.vector.tensor_scalar(satA[:, sl], canvas_ps[:], TANH_SAT,
                                        None, ALU.is_ge)
                xb = line[:].bitcast(dt.int32)
                nc.vector.tensor_scalar(mbA[:, sl], xb, mant_mask[:, 0:1],
                                        one_bits[:, 0:1],
                                        ALU.bitwise_and, ALU.bitwise_or)
                db = epi.tile([128, IMG], dt.int32, name="db")
                nc.vector.tensor_tensor(db[:], xb, mbA[:, sl], ALU.subtract)
                nc.vector.tensor_copy(efA[:, sl], db[:])
                nc.vector.tensor_scalar(efA[:, sl], efA[:, sl],
                                        0.6931471805599453 / (1 << 23),
                                        None, ALU.mult)
                nc.vector.tensor_scalar(snmA[:, sl], line[:], 1e-38, None,
                                        ALU.is_lt)
                nc.vector.tensor_scalar(upA[:, sl], line[:], -1.0, 1.0,
                                        ALU.mult, ALU.add)
                nc.vector.tensor_scalar(upA[:, sl], upA[:, sl], ULP_BELOW_1,
                                        None, ALU.max)

            pending = None
            for n in range(NB):
                T = Ts[n]
                # ring of 2 coefficient tiles; rows 32-127 zeroed once per
                # buffer (zeros-DMA), rows 0-31 re-filled per batch
                ft = fpool.tile([128, Ts[0] * 128], dt.bfloat16, name="ft")
                if n < 2:
                    nc.sync.dma_start(ft[32:128, :], z_in[:])
                nsplit = min(4, T)
                bounds = [round(i * T / nsplit) * 128
                          for i in range(nsplit + 1)]
                for i in range(nsplit):
                    nc.sync.dma_start(ft[0:32, bounds[i]:bounds[i + 1]],
                                      f_ins[n][:, bounds[i]:bounds[i + 1]])
                canvas_ps = canps.tile([128, IMG], dt.float32,
                                       name="canvas_ps")
                ch = 0
                while ch < T:
                    g = min(GRP, T - ch)
                    arg_ps = argps.tile([128, GRP * 2 * IMG], dt.float32,
                                        name="arg_ps")
                    for i in range(g):
                        nc.tensor.matmul(
                            arg_ps[:, i * 2 * IMG:(i + 1) * 2 * IMG],
                            ft[:, (ch + i) * 128:(ch + i + 1) * 128],
                            qt[:], start=True, stop=True)
                    gxy = gpool.tile([128, GRP * 2 * IMG], dt.bfloat16,
                                     name="gxy")
                    nc.scalar.activation(gxy[:, :g * 2 * IMG],
                                         arg_ps[:, :g * 2 * IMG], AF.Exp)
                    for i in range(g):
                        o = i * 2 * IMG
                        nc.tensor.matmul(
                            canvas_ps[:],
                            gxy[:, o:o + IMG], gxy[:, o + IMG:o + 2 * IMG],
                            start=(ch + i == 0), stop=(ch + i == T - 1))
                    ch += g
                    if pending is not None and ch >= GRP:
                        # previous batch's epilogue, after this batch's
                        # first group so tanh does not head-of-line block
                        emit_epilogue(*pending)
                        pending = None
                pending = (n, canvas_ps)
            emit_epilogue(*pending)

            # ---- deferred Ln phase (single table switch), batched over NB
            lnm = const_pool.tile([128, W4], dt.float32)
            lgm = const_pool.tile([128, W4], dt.float32)
            nc.scalar.activation(lnm[:], mbA[:].bitcast(dt.float32), AF.Ln)
            nc.scalar.activation(lgm[:], upA[:], AF.Ln)
            nc.vector.tensor_tensor(lnm[:], lnm[:], efA[:], ALU.add)
            nc.vector.copy_predicated(lnm[:], snmA[:], m100[:])
            nc.vector.tensor_tensor(lnm[:], imgt[:], lnm[:], ALU.mult)
            nc.vector.copy_predicated(lgm[:], satA[:], m100[:])
            nc.vector.tensor_tensor(lgm[:], imgmt[:], lgm[:], ALU.mult)
            nc.vector.tensor_tensor(lnm[:], lnm[:], lgm[:], ALU.add)
            nc.vector.tensor_reduce(outsb[:, 0:1], lnm[:],
                                    mybir.AxisListType.X, ALU.add)

            nc.sync.dma_start(out[:], outsb[:])
    nc.compile()
    return nc


_NC_CACHE = {}


def _get_nc(Ts):
    if Ts not in _NC_CACHE:
        _NC_CACHE[Ts] = _build_nc(Ts)
    return _NC_CACHE[Ts]


def make_in_maps(points, img, Ts, core_batches, samples):
    points = np.asarray(points, np.float32)
    img = np.asarray(img, np.float32)
    q24 = _build_q24()
    zros = np.zeros((96, Ts[0] * 128), _BF)
    in_maps = []
    for c in range(NCORES):
        bl = core_batches[c]
        pts = points[bl]
        im = {f"fs{n}": np.ascontiguousarray(
                  _build_f24(*samples[bl[n]], Ts[n] * 128))
              for n in range(NB)}
        im.update({
            "q24": q24,
            "zros": zros,
            "img": np.ascontiguousarray(img[bl]),
            "imgm": np.ascontiguousarray(1.0 - img[bl]),
            "ptsa": np.ascontiguousarray(pts[:, :P - 1, 0:2]),
            "ptsb": np.ascontiguousarray(pts[:, 1:, 0:2]),
        })
        in_maps.append(im)
    return in_maps


def combine_outputs(results):
    bce_tot = 0.0
    dist_tot = 0.0
    for r in results:
        o = np.asarray(r["out"], np.float64)
        bce_tot += o[:, 0].sum()
        dist_tot += o[:P - 1, NB:].sum()
    return np.float32((dist_tot - bce_tot) / N)


def kernel(points, img, _trace=False, _trace_kwargs=None):
    Ts, core_batches, samples = _schedule(points)
    nc = _get_nc(Ts)
    in_maps = make_in_maps(points, img, Ts, core_batches, samples)
    kw = {}
    if _trace:
        kw.update(trace=True, trace_cores=[0])
        if _trace_kwargs:
            kw.update(_trace_kwargs)
    res = run_bass_kernel_spmd(nc, in_maps, core_ids=list(range(NCORES)), **kw)
    out = combine_outputs(res.results)
    if _trace:
        return out, res
    return out


# revision 11
# speedup vs baseline: 4.0077x; 1.2464x over previous
"""Trainium2 Bass kernel for nn_Discriminator (histogram_binning / ridge).

Math (reference):
  For each batch n (N=32): interpolate P=128 points into M=(P-1)*181=22987
  line points (x,y,w); splat Gaussians g_x[m,s]=exp(-(x_m-s)^2/(2 w_m)),
  g_y[m,t]; canvas = g_x^T @ g_y  [128,128]; line = tanh(canvas);
  loss = sum(BCE(line, img))/N + sum(poly_sqrt(seg_len^2))/N.

Key optimization (adaptive segment resampling):
  The reference samples every segment at 181 points (one per ~0.37 px),
  but the splatted Gaussian has sigma = sqrt(w) >= 0.71 px; a trapezoid
  rule at spacing h needs only h <= ALPHA*sqrt(w) for aliasing error
  2*exp(-2*pi^2*w/h^2) (Poisson summation).  Per segment we place
  nseg+1 = ceil(len/(ALPHA*sqrt(w_min)))+1 trapezoid nodes spanning
  t in [0, 180/181]; interior amplitude r = 180/nseg, endpoint
  amplitude (r+1)/2 reproduces the reference's 181-term sum up to
  ~1e-3 relative.  This cuts M ~6.7x (23040 -> ~3400 per batch).
  Amplitudes fold into the Gaussian offset: c0x += ln(amp).

Device strategy (data-parallel over N, 4 batches per core, 8 cores;
batches assigned to the 4 slots by descending chunk count so every
slot's chunk count T_n is tight across cores):
  The Gaussian exponent arg[m,s] = c2[m]*s'^2 + c1[m]*s' + c0[m] (s'=s-64)
  is computed on the TensorEngine as a K=24 bf16 matmul (zero-padded to
  K=128): basis rows (s'^2 split into two exact bf16 rows, s', 1), each
  coefficient split into 3 bf16 levels (~25-bit mantissa).  A block-
  diagonal basis computes x-arg and y-arg in one matmul
  ([128,128m] lhsT x [128,256] rhs).  ScalarE applies Exp (PSUM->SBUF,
  bf16 out), and the canvas accumulates T_n chunk matmuls (K=128, bf16)
  in PSUM.  tanh/log/BCE epilogue per batch; final sums on host.
  Dead padding points use c0 = -200 so exp underflows to exactly 0.
"""
import sys
import types
import numpy as np
import ml_dtypes

# ---------------------------------------------------------------- constants
IMG = 128          # image size S
P = 128            # points per batch
N = 32             # batch
CMP = int(IMG * np.sqrt(2))            # 181
NCORES = 8
NB = N // NCORES                       # 4 batches per core
GRP = 6                                # arg chunks per Exp instruction
CENTER = 64.0
ALPHA = 3.5                            # resampling spacing, in sigmas

_d = np.arange(-IMG + 1, IMG)
X0 = float((_d ** 2 + (_d ** 2).T).mean().astype(np.float32))
C0 = float(X0 ** 0.5)
C1 = float(X0 ** (-0.5) / 2.0)
C2 = float(-(X0 ** (-1.5) / 8.0))
C3 = float(X0 ** (-2.5) / 16.0)

_BF = ml_dtypes.bfloat16

# XLA:CPU f32 tanh returns exactly 1.0 for x >= this (empirical, bit-exact);
# the reference's clip(log(1-line), -100) then yields -100 on those pixels.
TANH_SAT = float(np.uint32(1090516548).view(np.float32))  # 7.9988117
ULP_BELOW_1 = 5.960464477539063e-08  # 1 - nextafter(1, 0) in f32


def _install_ntff_hook():
    """bass_utils wants antenv.axon_hooks for trace=True under axon; the image
    lacks it. Provide it, backed by the ctypes shim in trn_agent_boot."""
    if 'antenv.axon_hooks' in sys.modules:
        return
    mod = types.ModuleType('antenv.axon_hooks')
    _h = [None]
    mod.set_axon_ntff_profile_hook = lambda h: _h.__setitem__(0, h)
    mod.get_axon_ntff_profile_hook = lambda: _h[0]
    sys.modules['antenv.axon_hooks'] = mod
    try:
        from trn_agent_boot.trn_boot import _ntff_profile_via_ctypes
        mod.set_axon_ntff_profile_hook(
            _ntff_profile_via_ctypes('/opt/axon/libaxon_pjrt.so'))
    except Exception:
        pass


_install_ntff_hook()

import concourse.bass as bass          # noqa: E402
import concourse.tile as tile          # noqa: E402
from concourse import bacc, mybir      # noqa: E402
from concourse.bass_utils import run_bass_kernel_spmd  # noqa: E402

dt = mybir.dt
AF = mybir.ActivationFunctionType
ALU = mybir.AluOpType


# ---------------------------------------------------------------- host prep
def _bf16_split3(x):
    h = x.astype(_BF).astype(np.float64)
    m = (x - h).astype(_BF).astype(np.float64)
    l = (x - h - m).astype(_BF).astype(np.float64)
    return h, m, l


def _build_q24():
    """Block-diagonal exact bf16 basis, zero-padded to K=128 rows (the PE's
    HAM clock-gate only counts full-K matmuls as activity)."""
    sprime = np.arange(IMG, dtype=np.float64) - CENTER
    s2 = sprime ** 2
    s2h = s2.astype(_BF).astype(np.float64)
    s2l = s2 - s2h
    qrows = [s2h, s2l, sprime, np.ones(IMG)]
    q = np.zeros((128, 2 * IMG))
    for base, off in ((0, 0), (12, IMG)):
        for lvl in range(3):
            for j in range(4):
                q[base + lvl * 4 + j, off:off + IMG] = qrows[j]
    return q.astype(_BF)


def _resample_batch(pts):
    """pts [P,3] f64 -> (xs, ys, ws, amps) trapezoid-node resampling of the
    reference's per-segment 181-point splat."""
    a = pts[:-1]                                   # [P-1, 3]
    b = pts[1:]
    tJ = (CMP - 1) / CMP
    e = a + (b - a) * tJ                           # last fine sample per seg
    seglen = np.sqrt(((b[:, :2] - a[:, :2]) ** 2).sum(-1)) * tJ
    wmin = np.minimum(a[:, 2], e[:, 2])
    nseg = np.clip(np.ceil(seglen / (ALPHA * np.sqrt(wmin))), 1,
                   CMP - 1).astype(int)
    xs_l, ys_l, ws_l, am_l = [], [], [], []
    for s in range(P - 1):
        ns = nseg[s]
        ti = np.arange(ns + 1) * (tJ / ns)
        r = (CMP - 1) / ns
        amp = np.full(ns + 1, r)
        amp[0] = amp[-1] = (r + 1) / 2
        xs_l.append(a[s, 0] + (b[s, 0] - a[s, 0]) * ti)
        ys_l.append(a[s, 1] + (b[s, 1] - a[s, 1]) * ti)
        ws_l.append(a[s, 2] + (b[s, 2] - a[s, 2]) * ti)
        am_l.append(amp)
    return (np.concatenate(xs_l), np.concatenate(ys_l),
            np.concatenate(ws_l), np.concatenate(am_l))


def _build_f24(xs, ys, ws, amps, width):
    """samples -> F [32, width] bf16 coefficient rows (padded with dead
    points whose exp underflows to exactly 0)."""
    m = len(xs)
    x = xs - CENTER
    y = ys - CENTER
    invw = 1.0 / ws
    c2 = -0.5 * invw
    c1x = x * invw
    c0x = -0.5 * x * x * invw + np.log(amps)
    c1y = y * invw
    c0y = -0.5 * y * y * invw

    F = np.zeros((32, width))
    for base, c1_, c0_ in ((0, c1x, c0x), (12, c1y, c0y)):
        splits = [_bf16_split3(c2), _bf16_split3(c2),
                  _bf16_split3(c1_), _bf16_split3(c0_)]
        for lvl in range(3):
            for j in range(4):
                F[base + lvl * 4 + j, :m] = splits[j][lvl]
    # dead padding: arg_x = arg_y = -200 -> exp == 0 exactly (f32 underflow)
    F[3, m:] = -200.0
    F[15, m:] = -200.0
    return F.astype(_BF)


def _schedule(points):
    """Resample all batches, assign them to (core, slot) so each slot's
    chunk count is tight. Returns per-slot widths Ts and per-core data."""
    points = np.asarray(points, np.float64)
    samples = [_resample_batch(points[b]) for b in range(N)]
    chunks = np.array([(len(s[0]) + 127) // 128 for s in samples])
    order = np.argsort(-chunks, kind='stable')
    Ts = []
    core_batches = [[] for _ in range(NCORES)]
    for n in range(NB):
        grp = order[n * NCORES:(n + 1) * NCORES]
        Ts.append(int(chunks[grp].max()))
        for c in range(NCORES):
            core_batches[c].append(int(grp[c]))
    return tuple(Ts), core_batches, samples


# ---------------------------------------------------------------- device
def _build_nc(Ts):
    nc = bacc.Bacc("TRN2", target_bir_lowering=False, debug=False,
                   enable_asserts=False, num_devices=NCORES)
    f_ins = [nc.dram_tensor(f"fs{n}", [32, Ts[n] * 128], dt.bfloat16,
                            kind="ExternalInput").ap()
             for n in range(NB)]
    z_in = nc.dram_tensor("zros", [96, Ts[0] * 128], dt.bfloat16,
                          kind="ExternalInput").ap()
    q_in = nc.dram_tensor("q24", [128, 2 * IMG], dt.bfloat16,
                          kind="ExternalInput").ap()
    img_in = nc.dram_tensor("img", [IMG, NB * IMG], dt.float32,
                            kind="ExternalInput").ap()
    pts_in = nc.dram_tensor("ptsab", [P - 1, 4 * NB], dt.float32,
                            kind="ExternalInput").ap()
    out = nc.dram_tensor("out", [128, 2 * NB], dt.float32,
                         kind="ExternalOutput").ap()
    W4 = NB * IMG

    with tile.TileContext(nc) as tc:
        with tc.tile_pool(name="const", bufs=1) as const_pool, \
             tc.tile_pool(name="fpool", bufs=2) as fpool, \
             tc.tile_pool(name="gpool", bufs=3) as gpool, \
             tc.tile_pool(name="small", bufs=2) as small, \
             tc.tile_pool(name="epi", bufs=2) as epi, \
             tc.tile_pool(name="argps", bufs=2, space="PSUM") as argps, \
             tc.tile_pool(name="canps", bufs=2, space="PSUM") as canps:

            qt = const_pool.tile([128, 2 * IMG], dt.bfloat16)
            nc.sync.dma_start(qt[:], q_in[:])
            outsb = const_pool.tile([128, 2 * NB], dt.float32)
            nc.vector.memset(outsb[:], 0.0)
            m100 = const_pool.tile([128, W4], dt.float32)
            nc.vector.memset(m100[:], -100.0)
            mant_mask = const_pool.tile([128, 1], dt.int32)
            nc.vector.memset(mant_mask[:], 0x007FFFFF)
            one_bits = const_pool.tile([128, 1], dt.int32)
            nc.vector.memset(one_bits[:], 0x3F800000)

            # dep-free warmups: preload the Exp table set and warm the PE
            # HAM clock gate while input DMAs are in flight (inputs are
            # uninitialized garbage; nothing reads the outputs)
            wsrc = const_pool.tile([128, 1], dt.float32, name="wsrc")
            nc.vector.memset(wsrc[:], 0.0)
            wdst = const_pool.tile([128, 1], dt.float32, name="wdst")
            nc.scalar.activation(wdst[:], wsrc[:], AF.Exp)
            wbf = const_pool.tile([128, 128], dt.bfloat16, name="wbf")
            nc.vector.memset(wbf[:], 1.0)
            wp = canps.tile([128, IMG], dt.float32, name="canvas_ps")
            for _ in range(25):
                nc.tensor.matmul(wp[:], wbf[:], wbf[:],
                                 start=True, stop=True)

            # persistent epilogue accumulators, written per batch slice
            mbA = const_pool.tile([128, W4], dt.int32)
            efA = const_pool.tile([128, W4], dt.float32)
            upA = const_pool.tile([128, W4], dt.float32)
            snmA = const_pool.tile([128, W4], dt.uint8)
            satA = const_pool.tile([128, W4], dt.uint8)

            def emit_epilogue(n, canvas_ps):
                # per-batch pre-Ln epilogue (tanh shares the Exp table set;
                # DVE work hides under the next batch's main loop)
                sl = slice(n * IMG, (n + 1) * IMG)
                line = epi.tile([128, IMG], dt.float32, name="line")
                nc.scalar.activation(line[:], canvas_ps[:], AF.Tanh)
                nc.vector.tensor_scalar(satA[:, sl], canvas_ps[:], TANH_SAT,
                                        None, ALU.is_ge)
                xb = line[:].bitcast(dt.int32)
                nc.vector.tensor_scalar(mbA[:, sl], xb, mant_mask[:, 0:1],
                                        one_bits[:, 0:1],
                                        ALU.bitwise_and, ALU.bitwise_or)
                db = epi.tile([128, IMG], dt.int32, name="db")
                nc.vector.tensor_tensor(db[:], xb, mbA[:, sl], ALU.subtract)
                nc.vector.tensor_copy(efA[:, sl], db[:])
                nc.vector.tensor_scalar(efA[:, sl], efA[:, sl],
                                        0.6931471805599453 / (1 << 23),
                                        None, ALU.mult)
                nc.vector.tensor_scalar(snmA[:, sl], line[:], 1e-38, None,
                                        ALU.is_lt)
                nc.vector.tensor_scalar(upA[:, sl], line[:], -1.0, 1.0,
                                        ALU.mult, ALU.add)
                nc.vector.tensor_scalar(upA[:, sl], upA[:, sl], ULP_BELOW_1,
                                        None, ALU.max)

            pending = None
            for n in range(NB):
                T = Ts[n]
                # ring of 2 coefficient tiles; rows 32-127 zeroed once per
                # buffer (zeros-DMA), rows 0-31 re-filled per batch
                ft = fpool.tile([128, Ts[0] * 128], dt.bfloat16, name="ft")
                nsplit = min(4, T)
                bounds = [round(i * T / nsplit) * 128
                          for i in range(nsplit + 1)]
                for i in range(nsplit):
                    nc.sync.dma_start(ft[0:32, bounds[i]:bounds[i + 1]],
                                      f_ins[n][:, bounds[i]:bounds[i + 1]])
                if n < 2:
                    nc.sync.dma_start(ft[32:128, :], z_in[:])
                kk = 32 if n == 0 else 128
                canvas_ps = canps.tile([128, IMG], dt.float32,
                                       name="canvas_ps")
                ch = 0
                while ch < T:
                    g = min(GRP, T - ch)
                    arg_ps = argps.tile([128, GRP * 2 * IMG], dt.float32,
                                        name="arg_ps")
                    for i in range(g):
                        nc.tensor.matmul(
                            arg_ps[:, i * 2 * IMG:(i + 1) * 2 * IMG],
                            ft[0:kk, (ch + i) * 128:(ch + i + 1) * 128],
                            qt[0:kk, :], start=True, stop=True)
                    gxy = gpool.tile([128, GRP * 2 * IMG], dt.bfloat16,
                                     name="gxy")
                    nc.scalar.activation(gxy[:, :g * 2 * IMG],
                                         arg_ps[:, :g * 2 * IMG], AF.Exp)
                    for i in range(g):
                        o = i * 2 * IMG
                        nc.tensor.matmul(
                            canvas_ps[:],
                            gxy[:, o:o + IMG], gxy[:, o + IMG:o + 2 * IMG],
                            start=(ch + i == 0), stop=(ch + i == T - 1))
                    ch += g
                    if pending is not None and ch >= GRP:
                        # previous batch's epilogue, after this batch's
                        # first group so tanh does not head-of-line block
                        emit_epilogue(*pending)
                        pending = None
                pending = (n, canvas_ps)
            emit_epilogue(*pending)

            # ---- tail inputs (kept off the startup DMA queues)
            imgt = const_pool.tile([128, W4], dt.float32)
            nc.sync.dma_start(imgt[:], img_in[:])
            imgmt = const_pool.tile([128, W4], dt.float32)
            nc.vector.tensor_scalar(imgmt[:], imgt[:], -1.0, 1.0,
                                    ALU.mult, ALU.add)

            # ---- distance term
            ptsab = const_pool.tile([P - 1, 4 * NB], dt.float32)
            nc.sync.dma_start(ptsab[:], pts_in[:])
            for n in range(NB):
                ta = ptsab[:, 4 * n:4 * n + 2]
                tb = ptsab[:, 4 * n + 2:4 * n + 4]
                dxy = epi.tile([P - 1, 2], dt.float32, name="dxy")
                nc.vector.tensor_tensor(dxy[:], tb, ta, ALU.subtract)
                nc.vector.tensor_tensor(dxy[:], dxy[:], dxy[:], ALU.mult)
                segsq = epi.tile([P - 1, 1], dt.float32, name="segsq")
                nc.vector.tensor_reduce(segsq[:], dxy[:],
                                        mybir.AxisListType.X, ALU.add)
                dx = epi.tile([P - 1, 1], dt.float32, name="dx")
                nc.vector.tensor_scalar(dx[:], segsq[:], -X0, None, ALU.add)
                poly = epi.tile([P - 1, 1], dt.float32, name="poly")
                nc.vector.tensor_scalar(poly[:], dx[:], C3, C2,
                                        ALU.mult, ALU.add)
                nc.vector.tensor_tensor(poly[:], poly[:], dx[:], ALU.mult)
                nc.vector.tensor_scalar(poly[:], poly[:], C1, None, ALU.add)
                nc.vector.tensor_tensor(poly[:], poly[:], dx[:], ALU.mult)
                nc.vector.tensor_scalar(outsb[:P - 1, NB + n:NB + n + 1],
                                        poly[:], C0, None, ALU.add)

            # ---- deferred Ln phase (single table switch), batched over NB
            lnm = const_pool.tile([128, W4], dt.float32)
            lgm = const_pool.tile([128, W4], dt.float32)
            nc.scalar.activation(lnm[:], mbA[:].bitcast(dt.float32), AF.Ln)
            nc.scalar.activation(lgm[:], upA[:], AF.Ln)
            nc.vector.tensor_tensor(lnm[:], lnm[:], efA[:], ALU.add)
            nc.vector.copy_predicated(lnm[:], snmA[:], m100[:])
            nc.vector.tensor_tensor(lnm[:], imgt[:], lnm[:], ALU.mult)
            nc.vector.copy_predicated(lgm[:], satA[:], m100[:])
            nc.vector.tensor_tensor(lgm[:], imgmt[:], lgm[:], ALU.mult)
            nc.vector.tensor_tensor(lnm[:], lnm[:], lgm[:], ALU.add)
            nc.vector.tensor_reduce(outsb[:, 0:1], lnm[:],
                                    mybir.AxisListType.X, ALU.add)

            nc.sync.dma_start(out[:], outsb[:])
    nc.compile()
    return nc


_NC_CACHE = {}


def _get_nc(Ts):
    if Ts not in _NC_CACHE:
        _NC_CACHE[Ts] = _build_nc(Ts)
    return _NC_CACHE[Ts]


def make_in_maps(points, img, Ts, core_batches, samples):
    points = np.asarray(points, np.float32)
    img = np.asarray(img, np.float32)
    q24 = _build_q24()
    zros = np.zeros((96, Ts[0] * 128), _BF)
    in_maps = []
    for c in range(NCORES):
        bl = core_batches[c]
        pts = points[bl]
        im = {f"fs{n}": np.ascontiguousarray(
                  _build_f24(*samples[bl[n]], Ts[n] * 128))
              for n in range(NB)}
        im.update({
            "q24": q24,
            "zros": zros,
            "img": np.ascontiguousarray(
                img[bl].transpose(1, 0, 2).reshape(IMG, NB * IMG)),
            "ptsab": np.ascontiguousarray(np.concatenate(
                [np.concatenate([pts[n, :P - 1, 0:2], pts[n, 1:, 0:2]],
                                axis=1) for n in range(NB)], axis=1)),
        })
        in_maps.append(im)
    return in_maps


def combine_outputs(results):
    bce_tot = 0.0
    dist_tot = 0.0
    for r in results:
        o = np.asarray(r["out"], np.float64)
        bce_tot += o[:, 0].sum()
        dist_tot += o[:P - 1, NB:].sum()
    return np.float32((dist_tot - bce_tot) / N)


def kernel(points, img, _trace=False, _trace_kwargs=None):
    Ts, core_batches, samples = _schedule(points)
    nc = _get_nc(Ts)
    in_maps = make_in_maps(points, img, Ts, core_batches, samples)
    kw = {}
    if _trace:
        kw.update(trace=True, trace_cores=[0])
        if _trace_kwargs:
            kw.update(_trace_kwargs)
    res = run_bass_kernel_spmd(nc, in_maps, core_ids=list(range(NCORES)), **kw)
    out = combine_outputs(res.results)
    if _trace:
        return out, res
    return out
